# revision 10
# baseline (speedup 1.0000x reference)
"""Trainium2 Bass kernel for nn_H_layer_85512798863503 (GNN message passing / GAT-style).

v6 strategy (self-contained; shapes hardcoded):
  - Shard edges across 8 cores by OWNER OF DST NODE; all segment reductions
    core-local -> no collectives.
  - Host BIN-PACKS nodes into 66 blocks of <=128 slots per core so every
    block holds exactly 1536 edge slots (12 tiles); uniform shapes, ~1.4% pad.
  - The three big streams (host-gathered src features, edge->slot one-hot,
    slot->edge one-hot) are FP8 e3m4 (exact for one-hots): 84MB -> ~44MB
    HBM per core.  Mixed-dtype matmuls: fp8 stationary x f16 moving.
  - P1 (per-node transform for dst tables + h output) hoisted upfront,
    batched 3 blocks per PSUM bank, batched DVE evacuations.
  - Edge pipeline is a 4-stage SOFTWARE PIPELINE over 6-tile chunks so no
    engine queue ever holds an instruction whose deps are younger than one
    chunk (no head-of-line stalls):
      A(c):  PE psE = xg@wsrc + onehot@dtab; ACT tanh->V, score-col copy;
             Pool tall = er*WaE
      B1(c-1): DVE ser = reduce(tall); Pool aa = sa + ser; ACT e1/e2 = exp
      B2(c-2): DVE e = max(e1,e2) -> V; DVE v1 = e * h_src (PSUM read;
             releases psE)
      C(c-3): PE psV += oh^T @ V;  per-block: DVE rc=1/sum_e, es=psV*rc;
             ACT es_er = psV*ivd
"""
import sys
if "/opt/trn_rl_repo" not in sys.path:
    sys.path.insert(0, "/opt/trn_rl_repo")

import numpy as np
import ml_dtypes

F16 = np.float16
F8 = ml_dtypes.float8_e3m4
EXPSHIFT = -5.54  # exp(a+EXPSHIFT): keeps e in f16 range; cancels in softmax ratio

N, E, DIN, HEAD, HD = 50000, 800000, 128, 4, 16
DOUT = HEAD * HD            # 64
NCORES = 8
NPC = N // NCORES           # 6250 nodes per core
NB = 128                    # dst slots per block
CAP = 1536                  # edge slots per block (12 tiles)
T = CAP // 128              # 12
CHUNK = 6                   # tiles per PSUM chunk (2 banks, 3 slots/half)
NEG = 0.01


def _blockdiag(w):
    m = np.zeros((DOUT, HEAD), np.float32)
    for h in range(HEAD):
        m[16 * h:16 * h + 16, h] = w
    return m


def _binpack(deg_core, nblk):
    """Greedy LPT: nodes (by degree desc) -> (block, slot). Returns
    block_of_node [NPC], slot_of_node [NPC] or (None, None) if infeasible."""
    order = np.argsort(-deg_core, kind="stable")
    bins_e = np.zeros(nblk, np.int64)
    bins_n = np.zeros(nblk, np.int64)
    blk = np.empty(NPC, np.int32)
    slot = np.empty(NPC, np.int32)
    for i in order:
        d = deg_core[i]
        cand = np.nonzero((bins_n < NB) & (bins_e + d <= CAP))[0]
        if len(cand) == 0:
            return None, None
        j = cand[np.argmin(bins_e[cand])]
        blk[i] = j
        slot[i] = bins_n[j]
        bins_e[j] += d
        bins_n[j] += 1
    return blk, slot


def _host_prep(x, src, dst, Ws, bs, Wd, bd, Wl, bl, Wa, ba):
    f32 = np.float32
    x = np.asarray(x, f32); src = np.asarray(src); dst = np.asarray(dst)

    # ---- weight folding ----
    WaS, WaD, WaE = Wa[0:16, 0], Wa[16:32, 0], Wa[32:48, 0]
    WaS_bd, WaD_bd = _blockdiag(WaS), _blockdiag(WaD)
    wsrc = np.concatenate([Ws, Wl @ WaS_bd, Wl], axis=1).astype(F16)         # [128,132]
    wnode = np.concatenate([Wl, Wd, Wl @ WaD_bd], axis=1).astype(F16)        # [128,132]
    bhr = np.tile(np.asarray(bl, f32)[None, :], (128, 1))                    # [128,64]
    bdst = np.concatenate([bs + bd, bl @ WaS_bd + bl @ WaD_bd + ba]).astype(f32)
    bdr = np.tile(bdst[None, :], (128, 1))                                   # [128,68]
    waer = np.tile(WaE[np.arange(DOUT) % 16][None, :], (128, 1)).astype(F16) # [128,64]
    blbf = np.tile(np.asarray(bl, F16)[None, :], (128, 1))                   # [128,64]

    x_f8 = x.astype(F8)
    x_bf = x.astype(F16)
    deg = np.bincount(dst, minlength=N).astype(np.int64)

    # ---- choose NBLK (uniform across cores; grow if packing infeasible) ----
    nblk = 66
    packs = None
    while True:
        packs = []
        ok = True
        for c in range(NCORES):
            dc = deg[c * NPC:(c + 1) * NPC]
            blk, slot = _binpack(dc, nblk)
            if blk is None:
                ok = False
                break
            packs.append((blk, slot))
        if ok:
            break
        nblk += 2
        assert nblk <= 80, "bin packing failed"

    NBLK = nblk
    NPAD = NBLK * NB
    STOT = NBLK * CAP

    shared = dict(wsrc=wsrc, wnode=wnode, waer=waer, bhr=bhr, bdr=bdr,
                  blbf=blbf)

    per_core_maps = []
    slot_nodes = []
    for c in range(NCORES):
        blk, slot = packs[c]                  # per local node
        nodes_l = np.arange(NPC)
        gslot = blk.astype(np.int64) * NB + slot  # node -> padded slot idx
        slot_node = np.full(NPAD, -1, np.int64)   # padded slot -> local node
        slot_node[gslot] = nodes_l

        ei = np.nonzero((dst >= c * NPC) & (dst < (c + 1) * NPC))[0]
        dl = dst[ei] - c * NPC
        eblk = blk[dl]
        edstloc = slot[dl]
        order = np.argsort(eblk, kind="stable")
        ks = eblk[order]
        rank = np.arange(len(ks)) - np.searchsorted(ks, ks)
        pos = ks.astype(np.int64) * CAP + rank    # edge slot in [0, STOT)
        assert rank.max() < CAP

        dstloc = np.full(STOT, -1, np.int16)
        dstloc[pos] = edstloc[order].astype(np.int16)

        # combined [xg | oht | oh] fp8, feature-major, per block contiguous:
        # layout [128, NBLK * 3 * CAP]; block b occupies cols [b*3C, (b+1)*3C)
        comb = np.zeros((128, NBLK * 3 * CAP), F8)
        # xg: src features feature-major
        xg_full = np.zeros((STOT, DIN), F8)
        xg_full[pos] = x_f8[src[ei][order]]
        xg3 = xg_full.reshape(NBLK, CAP, DIN)
        comb3 = comb.reshape(128, NBLK, 3, CAP)
        comb3[:, :, 0, :] = xg3.transpose(2, 0, 1)
        # oht[slot, e] = (dstloc == slot)
        sl = np.arange(NB, dtype=np.int16)
        oht = (dstloc.reshape(NBLK, CAP)[None, :, :] == sl[:, None, None])
        comb3[:, :, 1, :] = oht.astype(F8)
        # oh[e%128, (e//128)*128 + dstloc] = 1 (block-local cols)
        ohm = np.zeros((128, NBLK, CAP), F8)
        j = np.arange(STOT)
        valid = dstloc >= 0
        jj = j[valid]
        lc = jj % CAP
        ohm[lc % 128, jj // CAP, (lc // 128) * 128 + dstloc[valid]] = 1.0
        comb3[:, :, 2, :] = ohm

        # per-slot node features (permuted), zeros for empty slots
        xsl = np.zeros((128, NPAD), F16)
        xsl_t = x_bf[c * NPC:(c + 1) * NPC].T  # [128, NPC]
        xsl[:, gslot] = xsl_t[:, nodes_l]

        # 1/deg per slot
        ivd_flat = np.ones(NPAD, f32)
        dval = np.maximum(deg[c * NPC:(c + 1) * NPC], 1).astype(f32)
        ivd_flat[gslot] = 1.0 / dval
        ivd = np.ascontiguousarray(ivd_flat.reshape(NBLK, NB).T)  # [128, NBLK]

        m = dict(shared)
        m.update(xgoh=comb, ivd=ivd, xsl=xsl)
        per_core_maps.append(m)
        slot_nodes.append(slot_node)

    return NBLK, per_core_maps, slot_nodes


def _build_program(NBLK):
    import concourse.mybir as mybir
    import concourse.tile as tile
    from concourse import bacc
    from contextlib import ExitStack
    from collections import deque

    dt = mybir.dt
    Alu = mybir.AluOpType
    Act = mybir.ActivationFunctionType

    NPAD = NBLK * NB
    C3 = 3 * CAP

    nc = bacc.Bacc("TRN2", target_bir_lowering=False, debug=False,
                   num_devices=NCORES)

    xgoh = nc.dram_tensor("xgoh", [128, NBLK * C3], dt.float8e3,
                          kind="ExternalInput").ap()
    xsl = nc.dram_tensor("xsl", [128, NPAD], dt.float16, kind="ExternalInput").ap()
    wsrc = nc.dram_tensor("wsrc", [128, 132], dt.float16, kind="ExternalInput").ap()
    wnode = nc.dram_tensor("wnode", [128, 132], dt.float16, kind="ExternalInput").ap()
    waer = nc.dram_tensor("waer", [128, 64], dt.float16, kind="ExternalInput").ap()
    bhr = nc.dram_tensor("bhr", [128, 64], dt.float32, kind="ExternalInput").ap()
    bdr = nc.dram_tensor("bdr", [128, 68], dt.float32, kind="ExternalInput").ap()
    blbf = nc.dram_tensor("blbf", [128, 64], dt.float16, kind="ExternalInput").ap()
    ivd = nc.dram_tensor("ivd", [128, NBLK], dt.float32, kind="ExternalInput").ap()
    hout = nc.dram_tensor("hout", [NPAD, DOUT], dt.float16, kind="ExternalOutput").ap()
    esout = nc.dram_tensor("esout", [NPAD, 128], dt.float16, kind="ExternalOutput").ap()

    with tile.TileContext(nc) as tc:
        with ExitStack() as ctx:
            const = ctx.enter_context(tc.tile_pool(name="const", bufs=1))
            big = ctx.enter_context(tc.tile_pool(name="big", bufs=1))

            def cload(shape, dtyp, dram, tag):
                t = const.tile(shape, dtyp, tag=tag)
                nc.sync.dma_start(t[:], dram[:])
                return t

            wsrc_sb = cload([128, 132], dt.float16, wsrc, "wsrc")
            wnode_sb = cload([128, 132], dt.float16, wnode, "wnode")
            waer_sb = cload([128, 64], dt.float16, waer, "waer")
            bhr_sb = cload([128, 64], dt.float32, bhr, "bhr")
            bdr_sb = cload([128, 68], dt.float32, bdr, "bdr")
            blbf_sb = cload([128, 64], dt.float16, blbf, "blbf")
            ivd_sb = cload([128, NBLK], dt.float32, ivd, "ivd")
            xsl_sb = big.tile([128, NPAD], dt.float16, tag="xsl")
            nc.sync.dma_start(xsl_sb[:], xsl[:])

            ebias = const.tile([128, 1], dt.float32)
            nc.vector.memset(ebias[:], EXPSHIFT)

            dstTab = big.tile([128, NBLK * 132], dt.float16)
            dstTab3 = dstTab[:].rearrange("p (t c) -> p t c", c=132)
            esb = big.tile([128, NBLK * 128], dt.float16)
            es3 = esb[:].rearrange("p (t c) -> p t c", c=128)

            # constant bl columns of dstTab (cols 68:132 of each 132-block)
            nc.vector.tensor_copy(
                out=dstTab3[:, :, 68:132],
                in_=blbf_sb[:].rearrange("p c -> p () c")
                    .to_broadcast([128, NBLK, 64]))

            with tc.tile_pool(name="p1o", bufs=3) as p1o, \
                 tc.tile_pool(name="xg", bufs=6) as xgp, \
                 tc.tile_pool(name="vp", bufs=5) as vp, \
                 tc.tile_pool(name="tallp", bufs=3) as tallp, \
                 tc.tile_pool(name="scr", bufs=4) as scr, \
                 tc.tile_pool(name="psE", bufs=3, space="PSUM") as psEp, \
                 tc.tile_pool(name="psV", bufs=1, space="PSUM") as psVp:

                psVbig = psVp.tile([128, 512], dt.float32, tag="psv")
                p1bank = psVp.tile([128, 512], dt.float32, tag="p1b")

                # ---------------- P1 upfront: per-node transform ----------
                ngrp = (NBLK + 2) // 3
                for g in range(ngrp):
                    nb3 = min(3, NBLK - 3 * g)
                    ps = (psVbig if g % 2 == 0 else p1bank)[:]
                    for k in range(nb3):
                        b = 3 * g + k
                        nc.tensor.matmul(ps[:, k * 132:(k + 1) * 132],
                                         lhsT=xsl_sb[:, b * 128:(b + 1) * 128],
                                         rhs=wnode_sb[:], start=True, stop=True)
                    ps3 = ps[:, 0:nb3 * 132].rearrange("p (t c) -> p t c", c=132)
                    ht3 = p1o.tile([128, 3 * 64], dt.float16, tag="ht")
                    nc.vector.tensor_tensor(
                        out=ht3[:, 0:nb3 * 64].rearrange("p (t c) -> p t c", c=64),
                        in0=ps3[:, 0:nb3, 0:64],
                        in1=bhr_sb[:].rearrange("p c -> p () c")
                            .to_broadcast([128, nb3, 64]),
                        op=Alu.add)
                    nc.sync.dma_start(
                        hout[3 * g * 128:(3 * g + nb3) * 128, :]
                            .rearrange("(t p) c -> p t c", p=128),
                        ht3[:, 0:nb3 * 64].rearrange("p (t c) -> p t c", c=64))
                    nc.vector.tensor_tensor(
                        out=dstTab3[:, 3 * g:3 * g + nb3, 0:68],
                        in0=ps3[:, 0:nb3, 64:132],
                        in1=bdr_sb[:].rearrange("p c -> p () c")
                            .to_broadcast([128, nb3, 68]),
                        op=Alu.add)

                # ---------------- main edge loop: 4-stage pipeline --------
                NCH = T // CHUNK  # chunks per block
                nchunks = NBLK * NCH

                cstate = {}   # per-chunk tiles/aps shared across stages

                def stageA(ci):
                    b, ch = divmod(ci, NCH)
                    if ch == 0:
                        xg = xgp.tile([128, C3], dt.float8e3, tag="xg")
                        nc.sync.dma_start(xg[:], xgoh[:, b * C3:(b + 1) * C3])
                        cstate[('xg', b)] = xg
                    xg = cstate[('xg', b)]
                    dtab = dstTab3[:, b, :]
                    psE = psEp.tile([128, 1024], dt.float32)
                    for tt in range(CHUNK):
                        t = ch * CHUNK + tt
                        so = (tt // 3) * 512 + (tt % 3) * 132
                        nc.tensor.matmul(psE[:, so:so + 132],
                                         lhsT=xg[:, t * 128:(t + 1) * 128],
                                         rhs=wsrc_sb[:], start=True, stop=False)
                        nc.tensor.matmul(psE[:, so:so + 132],
                                         lhsT=xg[:, CAP + t * 128:CAP + (t + 1) * 128],
                                         rhs=dtab, start=False, stop=True)
                    pv = psE[:].rearrange("p (g r) -> p g r", r=512)[
                        :, :, 0:396].rearrange("p g (t c) -> p g t c", c=132)
                    Vc = vp.tile([128, CHUNK * 132], dt.float16, tag="V")
                    V3 = Vc[:].rearrange("p (t c) -> p t c", c=132)
                    V3g = Vc[:].rearrange("p (g t c) -> p g t c", g=2, c=132)
                    # evac: tanh(er) -> V, score cols -> sa
                    nc.scalar.activation(out=V3g[:, :, :, 64:128],
                                         in_=pv[:, :, :, 0:64], func=Act.Tanh)
                    sa = scr.tile([128, CHUNK * 4], dt.float32, tag="sa")
                    nc.scalar.activation(
                        out=sa[:].rearrange("p (g t c) -> p g t c", g=2, c=4),
                        in_=pv[:, :, :, 64:68], func=Act.Copy)
                    # tall = er * WaE
                    tall = tallp.tile([128, CHUNK * 64], dt.float16, tag="tall")
                    nc.gpsimd.tensor_tensor(
                        out=tall[:].rearrange("p (g t c) -> p g t c", g=2, c=64),
                        in0=V3g[:, :, :, 64:128],
                        in1=waer_sb[:].rearrange("p c -> p () () c")
                            .to_broadcast([128, 2, 3, 64]),
                        op=Alu.mult)
                    cstate[('pv', ci)] = pv
                    cstate[('V3', ci)] = V3
                    cstate[('V3g', ci)] = V3g
                    cstate[('sa', ci)] = sa
                    cstate[('tall', ci)] = tall

                def stageB1(ci):
                    tall = cstate.pop(('tall', ci))
                    sa = cstate.pop(('sa', ci))
                    ser = scr.tile([128, CHUNK * 4], dt.float32, tag="ser")
                    nc.vector.tensor_reduce(
                        out=ser[:].rearrange("p (t c) -> p t c", c=4),
                        in_=tall[:].rearrange("p (t h k) -> p t h k", h=4, k=16),
                        axis=mybir.AxisListType.X, op=Alu.add)
                    aa = scr.tile([128, CHUNK * 4], dt.float32, tag="aa")
                    nc.gpsimd.tensor_tensor(out=aa[:], in0=sa[:], in1=ser[:],
                                            op=Alu.add)
                    e1 = scr.tile([128, CHUNK * 4], dt.float32, tag="e1")
                    nc.scalar.activation(out=e1[:], in_=aa[:], func=Act.Exp,
                                         bias=ebias[:])
                    e2 = scr.tile([128, CHUNK * 4], dt.float32, tag="e2")
                    nc.scalar.activation(out=e2[:], in_=aa[:], func=Act.Exp,
                                         bias=ebias[:], scale=NEG)
                    cstate[('e1', ci)] = e1
                    cstate[('e2', ci)] = e2

                def stageB2(ci):
                    pv = cstate.pop(('pv', ci))
                    V3g = cstate.pop(('V3g', ci))
                    e1 = cstate.pop(('e1', ci))
                    e2 = cstate.pop(('e2', ci))
                    nc.vector.tensor_tensor(
                        out=V3g[:, :, :, 128:132],
                        in0=e1[:].rearrange("p (g t c) -> p g t c", g=2, c=4),
                        in1=e2[:].rearrange("p (g t c) -> p g t c", g=2, c=4),
                        op=Alu.max)
                    # v1 = e * h_src straight out of PSUM (releases psE)
                    for g2 in range(2):
                        nc.vector.tensor_tensor(
                            out=V3g[:, g2, :, 0:64]
                                .rearrange("p t (h k) -> p t h k", k=16),
                            in0=pv[:, g2, :, 68:132]
                                .rearrange("p t (h k) -> p t h k", k=16),
                            in1=V3g[:, g2, :, 128:132]
                                .to_broadcast([128, 3, 4, 16]),
                            op=Alu.mult)

                def norm(b):
                    par = (b % 2) * 132
                    psV = psVbig[:, par:par + 132]
                    rc = scr.tile([128, 4], dt.float32, tag="rc")
                    nc.vector.reciprocal(rc[:], psV[:, 128:132])
                    nc.vector.tensor_tensor(
                        out=es3[:, b, 0:64].rearrange("p (h k) -> p h k", k=16),
                        in0=psV[:, 0:64].rearrange("p (h k) -> p h k", k=16),
                        in1=rc[:].to_broadcast([128, 4, 16]),
                        op=Alu.mult)
                    nc.scalar.activation(
                        out=es3[:, b, 64:128], in_=psV[:, 64:128],
                        func=Act.Copy, scale=ivd_sb[:, b:b + 1])

                def stageC(ci):
                    b, ch = divmod(ci, NCH)
                    V3 = cstate.pop(('V3', ci))
                    xg = cstate[('xg', b)]
                    par = (b % 2) * 132
                    psV = psVbig[:, par:par + 132]
                    for tt in range(CHUNK):
                        t = ch * CHUNK + tt
                        nc.tensor.matmul(
                            psV,
                            lhsT=xg[:, 2 * CAP + t * 128:2 * CAP + (t + 1) * 128],
                            rhs=V3[:, tt, :],
                            start=(t == 0), stop=(t == T - 1))
                    if ch == NCH - 1:
                        cstate.pop(('xg', b))
                    elif ch == 0 and b > 0:
                        # normalize the PREVIOUS block (its psV finished one
                        # chunk ago; emitting here avoids a DVE head-of-line
                        # stall on the just-queued psV matmuls)
                        norm(b - 1)

                for ci in range(nchunks):
                    if ci >= 2:
                        stageB2(ci - 2)
                    if ci >= 3:
                        stageC(ci - 3)
                    if ci >= 1:
                        stageB1(ci - 1)
                    stageA(ci)
                # drain
                stageB1(nchunks - 1)
                for ci in (nchunks - 2, nchunks - 1):
                    stageB2(ci)
                for ci in (nchunks - 3, nchunks - 2, nchunks - 1):
                    stageC(ci)
                norm(NBLK - 1)

            nc.sync.dma_start(
                esout.rearrange("(t p) c -> p t c", p=128),
                es3)

    nc.compile()
    return nc


_CACHE = {}


def _get_program(NBLK):
    if NBLK not in _CACHE:
        _CACHE[NBLK] = _build_program(NBLK)
    return _CACHE[NBLK]


def _install_ntff_shim():
    """The image's antenv lacks axon_hooks; supply it so bass_utils can
    drive NTFF profiling through libaxon_pjrt."""
    import types
    import antenv
    if "antenv.axon_hooks" in sys.modules:
        return
    mod = types.ModuleType("antenv.axon_hooks")
    mod._hook = None
    mod.set_axon_ntff_profile_hook = lambda h: setattr(mod, "_hook", h)
    mod.get_axon_ntff_profile_hook = lambda: mod._hook
    sys.modules["antenv.axon_hooks"] = mod
    antenv.axon_hooks = mod
    from trn_agent_boot.trn_boot import _ntff_profile_via_ctypes
    mod._hook = _ntff_profile_via_ctypes("/opt/axon/libaxon_pjrt.so")


def run(inputs, trace=False, trace_kwargs=None):
    """Build + run; returns (edge_s, out, h) plus the raw BassKernelResults."""
    from concourse.bass_utils import run_bass_kernel_spmd

    NBLK, per_core_maps, slot_nodes = _host_prep(**inputs)
    nc = _get_program(NBLK)
    in_maps = [{k: np.ascontiguousarray(v) for k, v in m.items()}
               for m in per_core_maps]
    kw = {}
    if trace:
        _install_ntff_shim()
        kw = dict(trace=True, **(trace_kwargs or {}))
    res = run_bass_kernel_spmd(nc, in_maps, core_ids=list(range(NCORES)), **kw)

    edge_s = np.empty((N, DOUT), np.float32)
    out = np.empty((N, DOUT), np.float32)
    h = np.empty((N, DOUT), np.float32)
    for c in range(NCORES):
        r = res.results[c]
        es = np.asarray(r["esout"], np.float32)
        hh = np.asarray(r["hout"], np.float32)
        sn = slot_nodes[c]
        valid = sn >= 0
        gids = c * NPC + sn[valid]
        out[gids] = es[valid, 0:64]
        edge_s[gids] = es[valid, 64:128]
        h[gids] = hh[valid]
    return (edge_s, out, h), res


def kernel(**inputs):
    (edge_s, out, h), _ = run(inputs)
    return (edge_s, out, h)


# revision 12
# speedup vs baseline: 1.0009x; 1.0009x over previous
"""Trainium2 Bass kernel for nn_H_layer_85512798863503 (GNN message passing / GAT-style).

v6 strategy (self-contained; shapes hardcoded):
  - Shard edges across 8 cores by OWNER OF DST NODE; all segment reductions
    core-local -> no collectives.
  - Host BIN-PACKS nodes into 66 blocks of <=128 slots per core so every
    block holds exactly 1536 edge slots (12 tiles); uniform shapes, ~1.4% pad.
  - The three big streams (host-gathered src features, edge->slot one-hot,
    slot->edge one-hot) are FP8 e3m4 (exact for one-hots): 84MB -> ~44MB
    HBM per core.  Mixed-dtype matmuls: fp8 stationary x f16 moving.
  - P1 (per-node transform for dst tables + h output) hoisted upfront,
    batched 3 blocks per PSUM bank, batched DVE evacuations.
  - Edge pipeline is a 4-stage SOFTWARE PIPELINE over 6-tile chunks so no
    engine queue ever holds an instruction whose deps are younger than one
    chunk (no head-of-line stalls):
      A(c):  PE psE = xg@wsrc + onehot@dtab; ACT tanh->V, score-col copy;
             Pool tall = er*WaE
      B1(c-1): DVE ser = reduce(tall); Pool aa = sa + ser; ACT e1/e2 = exp
      B2(c-2): DVE e = max(e1,e2) -> V; DVE v1 = e * h_src (PSUM read;
             releases psE)
      C(c-3): PE psV += oh^T @ V;  per-block: DVE rc=1/sum_e, es=psV*rc;
             ACT es_er = psV*ivd
"""
import sys
if "/opt/trn_rl_repo" not in sys.path:
    sys.path.insert(0, "/opt/trn_rl_repo")

import numpy as np
import ml_dtypes

F16 = np.float16
F8 = ml_dtypes.float8_e3m4
EXPSHIFT = -5.54  # exp(a+EXPSHIFT): keeps e in f16 range; cancels in softmax ratio

N, E, DIN, HEAD, HD = 50000, 800000, 128, 4, 16
DOUT = HEAD * HD            # 64
NCORES = 8
NPC = N // NCORES           # 6250 nodes per core
NB = 128                    # dst slots per block
CAP = 1536                  # edge slots per block (12 tiles)
T = CAP // 128              # 12
CHUNK = 6                   # tiles per PSUM chunk (2 banks, 3 slots/half)
NEG = 0.01


def _blockdiag(w):
    m = np.zeros((DOUT, HEAD), np.float32)
    for h in range(HEAD):
        m[16 * h:16 * h + 16, h] = w
    return m


def _binpack(deg_core, nblk):
    """Greedy LPT: nodes (by degree desc) -> (block, slot). Returns
    block_of_node [NPC], slot_of_node [NPC] or (None, None) if infeasible."""
    order = np.argsort(-deg_core, kind="stable")
    bins_e = np.zeros(nblk, np.int64)
    bins_n = np.zeros(nblk, np.int64)
    blk = np.empty(NPC, np.int32)
    slot = np.empty(NPC, np.int32)
    for i in order:
        d = deg_core[i]
        cand = np.nonzero((bins_n < NB) & (bins_e + d <= CAP))[0]
        if len(cand) == 0:
            return None, None
        j = cand[np.argmin(bins_e[cand])]
        blk[i] = j
        slot[i] = bins_n[j]
        bins_e[j] += d
        bins_n[j] += 1
    return blk, slot


def _host_prep(x, src, dst, Ws, bs, Wd, bd, Wl, bl, Wa, ba):
    f32 = np.float32
    x = np.asarray(x, f32); src = np.asarray(src); dst = np.asarray(dst)

    # ---- weight folding ----
    WaS, WaD, WaE = Wa[0:16, 0], Wa[16:32, 0], Wa[32:48, 0]
    WaS_bd, WaD_bd = _blockdiag(WaS), _blockdiag(WaD)
    wsrc = np.concatenate([Ws, Wl @ WaS_bd, Wl], axis=1).astype(F16)         # [128,132]
    wnode = np.concatenate([Wl, Wd, Wl @ WaD_bd], axis=1).astype(F16)        # [128,132]
    bhr = np.tile(np.asarray(bl, f32)[None, :], (128, 1))                    # [128,64]
    bdst = np.concatenate([bs + bd, bl @ WaS_bd + bl @ WaD_bd + ba]).astype(f32)
    bdr = np.tile(bdst[None, :], (128, 1))                                   # [128,68]
    waer = np.tile(WaE[np.arange(DOUT) % 16][None, :], (128, 1)).astype(F16) # [128,64]
    blbf = np.tile(np.asarray(bl, F16)[None, :], (128, 1))                   # [128,64]

    x_f8 = x.astype(F8)
    x_bf = x.astype(F16)
    deg = np.bincount(dst, minlength=N).astype(np.int64)

    # ---- choose NBLK (uniform across cores; grow if packing infeasible) ----
    nblk = 66
    packs = None
    while True:
        packs = []
        ok = True
        for c in range(NCORES):
            dc = deg[c * NPC:(c + 1) * NPC]
            blk, slot = _binpack(dc, nblk)
            if blk is None:
                ok = False
                break
            packs.append((blk, slot))
        if ok:
            break
        nblk += 2
        assert nblk <= 80, "bin packing failed"

    NBLK = nblk
    NPAD = NBLK * NB
    STOT = NBLK * CAP

    shared = dict(wsrc=wsrc, wnode=wnode, waer=waer, bhr=bhr, bdr=bdr,
                  blbf=blbf)

    per_core_maps = []
    slot_nodes = []
    for c in range(NCORES):
        blk, slot = packs[c]                  # per local node
        nodes_l = np.arange(NPC)
        gslot = blk.astype(np.int64) * NB + slot  # node -> padded slot idx
        slot_node = np.full(NPAD, -1, np.int64)   # padded slot -> local node
        slot_node[gslot] = nodes_l

        ei = np.nonzero((dst >= c * NPC) & (dst < (c + 1) * NPC))[0]
        dl = dst[ei] - c * NPC
        eblk = blk[dl]
        edstloc = slot[dl]
        order = np.argsort(eblk, kind="stable")
        ks = eblk[order]
        rank = np.arange(len(ks)) - np.searchsorted(ks, ks)
        pos = ks.astype(np.int64) * CAP + rank    # edge slot in [0, STOT)
        assert rank.max() < CAP

        dstloc = np.full(STOT, -1, np.int16)
        dstloc[pos] = edstloc[order].astype(np.int16)

        # combined [xg | oht | oh] fp8, feature-major, per block contiguous:
        # layout [128, NBLK * 3 * CAP]; block b occupies cols [b*3C, (b+1)*3C)
        comb = np.zeros((128, NBLK * 3 * CAP), F8)
        # xg: src features feature-major
        xg_full = np.zeros((STOT, DIN), F8)
        xg_full[pos] = x_f8[src[ei][order]]
        xg3 = xg_full.reshape(NBLK, CAP, DIN)
        comb3 = comb.reshape(128, NBLK, 3, CAP)
        comb3[:, :, 0, :] = xg3.transpose(2, 0, 1)
        # oht[slot, e] = (dstloc == slot)
        sl = np.arange(NB, dtype=np.int16)
        oht = (dstloc.reshape(NBLK, CAP)[None, :, :] == sl[:, None, None])
        comb3[:, :, 1, :] = oht.astype(F8)
        # oh[e%128, (e//128)*128 + dstloc] = 1 (block-local cols)
        ohm = np.zeros((128, NBLK, CAP), F8)
        j = np.arange(STOT)
        valid = dstloc >= 0
        jj = j[valid]
        lc = jj % CAP
        ohm[lc % 128, jj // CAP, (lc // 128) * 128 + dstloc[valid]] = 1.0
        comb3[:, :, 2, :] = ohm

        # per-slot node features (permuted), zeros for empty slots
        xsl = np.zeros((128, NPAD), F16)
        xsl_t = x_bf[c * NPC:(c + 1) * NPC].T  # [128, NPC]
        xsl[:, gslot] = xsl_t[:, nodes_l]

        # 1/deg per slot
        ivd_flat = np.ones(NPAD, f32)
        dval = np.maximum(deg[c * NPC:(c + 1) * NPC], 1).astype(f32)
        ivd_flat[gslot] = 1.0 / dval
        ivd = np.ascontiguousarray(ivd_flat.reshape(NBLK, NB).T)  # [128, NBLK]

        m = dict(shared)
        m.update(xgoh=comb, ivd=ivd, xsl=xsl)
        per_core_maps.append(m)
        slot_nodes.append(slot_node)

    return NBLK, per_core_maps, slot_nodes


def _build_program(NBLK):
    import concourse.mybir as mybir
    import concourse.tile as tile
    from concourse import bacc
    from contextlib import ExitStack
    from collections import deque

    dt = mybir.dt
    Alu = mybir.AluOpType
    Act = mybir.ActivationFunctionType

    NPAD = NBLK * NB
    C3 = 3 * CAP

    nc = bacc.Bacc("TRN2", target_bir_lowering=False, debug=False,
                   num_devices=NCORES)

    xgoh = nc.dram_tensor("xgoh", [128, NBLK * C3], dt.float8e3,
                          kind="ExternalInput").ap()
    xsl = nc.dram_tensor("xsl", [128, NPAD], dt.float16, kind="ExternalInput").ap()
    wsrc = nc.dram_tensor("wsrc", [128, 132], dt.float16, kind="ExternalInput").ap()
    wnode = nc.dram_tensor("wnode", [128, 132], dt.float16, kind="ExternalInput").ap()
    waer = nc.dram_tensor("waer", [128, 64], dt.float16, kind="ExternalInput").ap()
    bhr = nc.dram_tensor("bhr", [128, 64], dt.float32, kind="ExternalInput").ap()
    bdr = nc.dram_tensor("bdr", [128, 68], dt.float32, kind="ExternalInput").ap()
    blbf = nc.dram_tensor("blbf", [128, 64], dt.float16, kind="ExternalInput").ap()
    ivd = nc.dram_tensor("ivd", [128, NBLK], dt.float32, kind="ExternalInput").ap()
    hout = nc.dram_tensor("hout", [NPAD, DOUT], dt.float16, kind="ExternalOutput").ap()
    esout = nc.dram_tensor("esout", [NPAD, 128], dt.float16, kind="ExternalOutput").ap()

    with tile.TileContext(nc) as tc:
        with ExitStack() as ctx:
            const = ctx.enter_context(tc.tile_pool(name="const", bufs=1))
            big = ctx.enter_context(tc.tile_pool(name="big", bufs=1))

            def cload(shape, dtyp, dram, tag):
                t = const.tile(shape, dtyp, tag=tag)
                nc.sync.dma_start(t[:], dram[:])
                return t

            wsrc_sb = cload([128, 132], dt.float16, wsrc, "wsrc")
            wnode_sb = cload([128, 132], dt.float16, wnode, "wnode")
            waer_sb = cload([128, 64], dt.float16, waer, "waer")
            bhr_sb = cload([128, 64], dt.float32, bhr, "bhr")
            bdr_sb = cload([128, 68], dt.float32, bdr, "bdr")
            blbf_sb = cload([128, 64], dt.float16, blbf, "blbf")
            ivd_sb = cload([128, NBLK], dt.float32, ivd, "ivd")
            xsl_sb = big.tile([128, NPAD], dt.float16, tag="xsl")
            nc.sync.dma_start(xsl_sb[:], xsl[:])

            ebias = const.tile([128, 1], dt.float32)
            nc.vector.memset(ebias[:], EXPSHIFT)

            dstTab = big.tile([128, NBLK * 132], dt.float16)
            dstTab3 = dstTab[:].rearrange("p (t c) -> p t c", c=132)
            esb = big.tile([128, NBLK * 128], dt.float16)
            es3 = esb[:].rearrange("p (t c) -> p t c", c=128)

            # constant bl columns of dstTab (cols 68:132 of each 132-block)
            nc.vector.tensor_copy(
                out=dstTab3[:, :, 68:132],
                in_=blbf_sb[:].rearrange("p c -> p () c")
                    .to_broadcast([128, NBLK, 64]))

            with tc.tile_pool(name="p1o", bufs=3) as p1o, \
                 tc.tile_pool(name="xg", bufs=6) as xgp, \
                 tc.tile_pool(name="vp", bufs=5) as vp, \
                 tc.tile_pool(name="tallp", bufs=3) as tallp, \
                 tc.tile_pool(name="scr", bufs=4) as scr, \
                 tc.tile_pool(name="psE", bufs=3, space="PSUM") as psEp, \
                 tc.tile_pool(name="psV", bufs=1, space="PSUM") as psVp:

                psVbig = psVp.tile([128, 512], dt.float32, tag="psv")
                p1bank = psVp.tile([128, 512], dt.float32, tag="p1b")

                # ---------------- P1 upfront: per-node transform ----------
                ngrp = (NBLK + 2) // 3
                for g in range(ngrp):
                    nb3 = min(3, NBLK - 3 * g)
                    ps = (psVbig if g % 2 == 0 else p1bank)[:]
                    for k in range(nb3):
                        b = 3 * g + k
                        nc.tensor.matmul(ps[:, k * 132:(k + 1) * 132],
                                         lhsT=xsl_sb[:, b * 128:(b + 1) * 128],
                                         rhs=wnode_sb[:], start=True, stop=True)
                    ps3 = ps[:, 0:nb3 * 132].rearrange("p (t c) -> p t c", c=132)
                    ht3 = p1o.tile([128, 3 * 64], dt.float16, tag="ht")
                    nc.vector.tensor_tensor(
                        out=ht3[:, 0:nb3 * 64].rearrange("p (t c) -> p t c", c=64),
                        in0=ps3[:, 0:nb3, 0:64],
                        in1=bhr_sb[:].rearrange("p c -> p () c")
                            .to_broadcast([128, nb3, 64]),
                        op=Alu.add)
                    nc.sync.dma_start(
                        hout[3 * g * 128:(3 * g + nb3) * 128, :]
                            .rearrange("(t p) c -> p t c", p=128),
                        ht3[:, 0:nb3 * 64].rearrange("p (t c) -> p t c", c=64))
                    nc.vector.tensor_tensor(
                        out=dstTab3[:, 3 * g:3 * g + nb3, 0:68],
                        in0=ps3[:, 0:nb3, 64:132],
                        in1=bdr_sb[:].rearrange("p c -> p () c")
                            .to_broadcast([128, nb3, 68]),
                        op=Alu.add)

                # ---------------- main edge loop: 4-stage pipeline --------
                NCH = T // CHUNK  # chunks per block
                nchunks = NBLK * NCH

                cstate = {}   # per-chunk tiles/aps shared across stages

                def stageA(ci):
                    b, ch = divmod(ci, NCH)
                    if ch == 0:
                        xg = xgp.tile([128, C3], dt.float8e3, tag="xg")
                        nc.sync.dma_start(xg[:], xgoh[:, b * C3:(b + 1) * C3])
                        cstate[('xg', b)] = xg
                    xg = cstate[('xg', b)]
                    dtab = dstTab3[:, b, :]
                    psE = psEp.tile([128, 1024], dt.float32)
                    for tt in range(CHUNK):
                        t = ch * CHUNK + tt
                        so = (tt // 3) * 512 + (tt % 3) * 132
                        nc.tensor.matmul(psE[:, so:so + 132],
                                         lhsT=xg[:, t * 128:(t + 1) * 128],
                                         rhs=wsrc_sb[:], start=True, stop=False)
                        nc.tensor.matmul(psE[:, so:so + 132],
                                         lhsT=xg[:, CAP + t * 128:CAP + (t + 1) * 128],
                                         rhs=dtab, start=False, stop=True)
                    pv = psE[:].rearrange("p (g r) -> p g r", r=512)[
                        :, :, 0:396].rearrange("p g (t c) -> p g t c", c=132)
                    Vc = vp.tile([128, CHUNK * 132], dt.float16, tag="V")
                    V3 = Vc[:].rearrange("p (t c) -> p t c", c=132)
                    V3g = Vc[:].rearrange("p (g t c) -> p g t c", g=2, c=132)
                    # evac: tanh(er) -> V, score cols -> sa, h_src -> hlS
                    # (psE is fully evacuated in stage A so the PSUM chunk
                    #  recycles without waiting for the score tail)
                    nc.scalar.activation(out=V3g[:, :, :, 64:128],
                                         in_=pv[:, :, :, 0:64], func=Act.Tanh)
                    sa = scr.tile([128, CHUNK * 4], dt.float32, tag="sa")
                    nc.scalar.activation(
                        out=sa[:].rearrange("p (g t c) -> p g t c", g=2, c=4),
                        in_=pv[:, :, :, 64:68], func=Act.Copy)
                    hlS = tallp.tile([128, CHUNK * 64], dt.float16, tag="hlS")
                    hl3 = hlS[:].rearrange("p (g t c) -> p g t c", g=2, c=64)
                    nc.scalar.activation(out=hl3[:, 0, :, :],
                                         in_=pv[:, 0, :, 68:132], func=Act.Copy)
                    nc.vector.tensor_copy(out=hl3[:, 1, :, :],
                                          in_=pv[:, 1, :, 68:132])
                    # tall = er * WaE
                    tall = tallp.tile([128, CHUNK * 64], dt.float16, tag="tall")
                    nc.gpsimd.tensor_tensor(
                        out=tall[:].rearrange("p (g t c) -> p g t c", g=2, c=64),
                        in0=V3g[:, :, :, 64:128],
                        in1=waer_sb[:].rearrange("p c -> p () () c")
                            .to_broadcast([128, 2, 3, 64]),
                        op=Alu.mult)
                    cstate[('V3', ci)] = V3
                    cstate[('V3g', ci)] = V3g
                    cstate[('hlS', ci)] = hlS
                    cstate[('sa', ci)] = sa
                    cstate[('tall', ci)] = tall

                def stageB1(ci):
                    tall = cstate.pop(('tall', ci))
                    sa = cstate.pop(('sa', ci))
                    ser = scr.tile([128, CHUNK * 4], dt.float32, tag="ser")
                    nc.vector.tensor_reduce(
                        out=ser[:].rearrange("p (t c) -> p t c", c=4),
                        in_=tall[:].rearrange("p (t h k) -> p t h k", h=4, k=16),
                        axis=mybir.AxisListType.X, op=Alu.add)
                    aa = scr.tile([128, CHUNK * 4], dt.float32, tag="aa")
                    nc.gpsimd.tensor_tensor(out=aa[:], in0=sa[:], in1=ser[:],
                                            op=Alu.add)
                    e1 = scr.tile([128, CHUNK * 4], dt.float32, tag="e1")
                    nc.scalar.activation(out=e1[:], in_=aa[:], func=Act.Exp,
                                         bias=ebias[:])
                    e2 = scr.tile([128, CHUNK * 4], dt.float32, tag="e2")
                    nc.scalar.activation(out=e2[:], in_=aa[:], func=Act.Exp,
                                         bias=ebias[:], scale=NEG)
                    cstate[('e1', ci)] = e1
                    cstate[('e2', ci)] = e2

                def stageB2(ci):
                    V3g = cstate.pop(('V3g', ci))
                    hlS = cstate.pop(('hlS', ci))
                    hl3 = hlS[:].rearrange("p (g t c) -> p g t c", g=2, c=64)
                    e1 = cstate.pop(('e1', ci))
                    e2 = cstate.pop(('e2', ci))
                    nc.vector.tensor_tensor(
                        out=V3g[:, :, :, 128:132],
                        in0=e1[:].rearrange("p (g t c) -> p g t c", g=2, c=4),
                        in1=e2[:].rearrange("p (g t c) -> p g t c", g=2, c=4),
                        op=Alu.max)
                    # v1 = e * h_src (SBUF f16; split across Pool and DVE)
                    nc.gpsimd.tensor_tensor(
                        out=V3g[:, 0, :, 0:64]
                            .rearrange("p t (h k) -> p t h k", k=16),
                        in0=hl3[:, 0, :, :].rearrange("p t (h k) -> p t h k", k=16),
                        in1=V3g[:, 0, :, 128:132]
                            .to_broadcast([128, 3, 4, 16]),
                        op=Alu.mult)
                    nc.vector.tensor_tensor(
                        out=V3g[:, 1, :, 0:64]
                            .rearrange("p t (h k) -> p t h k", k=16),
                        in0=hl3[:, 1, :, :].rearrange("p t (h k) -> p t h k", k=16),
                        in1=V3g[:, 1, :, 128:132]
                            .to_broadcast([128, 3, 4, 16]),
                        op=Alu.mult)

                def norm(b):
                    par = (b % 2) * 132
                    psV = psVbig[:, par:par + 132]
                    rc = scr.tile([128, 4], dt.float32, tag="rc")
                    nc.vector.reciprocal(rc[:], psV[:, 128:132])
                    nc.vector.tensor_tensor(
                        out=es3[:, b, 0:64].rearrange("p (h k) -> p h k", k=16),
                        in0=psV[:, 0:64].rearrange("p (h k) -> p h k", k=16),
                        in1=rc[:].to_broadcast([128, 4, 16]),
                        op=Alu.mult)
                    nc.scalar.activation(
                        out=es3[:, b, 64:128], in_=psV[:, 64:128],
                        func=Act.Copy, scale=ivd_sb[:, b:b + 1])

                def stageC(ci):
                    b, ch = divmod(ci, NCH)
                    V3 = cstate.pop(('V3', ci))
                    xg = cstate[('xg', b)]
                    par = (b % 2) * 132
                    psV = psVbig[:, par:par + 132]
                    for tt in range(CHUNK):
                        t = ch * CHUNK + tt
                        nc.tensor.matmul(
                            psV,
                            lhsT=xg[:, 2 * CAP + t * 128:2 * CAP + (t + 1) * 128],
                            rhs=V3[:, tt, :],
                            start=(t == 0), stop=(t == T - 1))
                    if ch == NCH - 1:
                        cstate.pop(('xg', b))
                    elif ch == 0 and b > 0:
                        # normalize the PREVIOUS block (its psV finished one
                        # chunk ago; emitting here avoids a DVE head-of-line
                        # stall on the just-queued psV matmuls)
                        norm(b - 1)

                for ci in range(nchunks):
                    if ci >= 2:
                        stageB2(ci - 2)
                    if ci >= 3:
                        stageC(ci - 3)
                    if ci >= 1:
                        stageB1(ci - 1)
                    stageA(ci)
                # drain
                stageB1(nchunks - 1)
                for ci in (nchunks - 2, nchunks - 1):
                    stageB2(ci)
                for ci in (nchunks - 3, nchunks - 2, nchunks - 1):
                    stageC(ci)
                norm(NBLK - 1)

            nc.sync.dma_start(
                esout.rearrange("(t p) c -> p t c", p=128),
                es3)

    nc.compile()
    return nc


_CACHE = {}


def _get_program(NBLK):
    if NBLK not in _CACHE:
        _CACHE[NBLK] = _build_program(NBLK)
    return _CACHE[NBLK]


def _install_ntff_shim():
    """The image's antenv lacks axon_hooks; supply it so bass_utils can
    drive NTFF profiling through libaxon_pjrt."""
    import types
    import antenv
    if "antenv.axon_hooks" in sys.modules:
        return
    mod = types.ModuleType("antenv.axon_hooks")
    mod._hook = None
    mod.set_axon_ntff_profile_hook = lambda h: setattr(mod, "_hook", h)
    mod.get_axon_ntff_profile_hook = lambda: mod._hook
    sys.modules["antenv.axon_hooks"] = mod
    antenv.axon_hooks = mod
    from trn_agent_boot.trn_boot import _ntff_profile_via_ctypes
    mod._hook = _ntff_profile_via_ctypes("/opt/axon/libaxon_pjrt.so")


def run(inputs, trace=False, trace_kwargs=None):
    """Build + run; returns (edge_s, out, h) plus the raw BassKernelResults."""
    from concourse.bass_utils import run_bass_kernel_spmd

    NBLK, per_core_maps, slot_nodes = _host_prep(**inputs)
    nc = _get_program(NBLK)
    in_maps = [{k: np.ascontiguousarray(v) for k, v in m.items()}
               for m in per_core_maps]
    kw = {}
    if trace:
        _install_ntff_shim()
        kw = dict(trace=True, **(trace_kwargs or {}))
    res = run_bass_kernel_spmd(nc, in_maps, core_ids=list(range(NCORES)), **kw)

    edge_s = np.empty((N, DOUT), np.float32)
    out = np.empty((N, DOUT), np.float32)
    h = np.empty((N, DOUT), np.float32)
    for c in range(NCORES):
        r = res.results[c]
        es = np.asarray(r["esout"], np.float32)
        hh = np.asarray(r["hout"], np.float32)
        sn = slot_nodes[c]
        valid = sn >= 0
        gids = c * NPC + sn[valid]
        out[gids] = es[valid, 0:64]
        edge_s[gids] = es[valid, 64:128]
        h[gids] = hh[valid]
    return (edge_s, out, h), res


def kernel(**inputs):
    (edge_s, out, h), _ = run(inputs)
    return (edge_s, out, h)
